# revision 7
# baseline (speedup 1.0000x reference)
"""Biquad lowpass filter (torchaudio lowpass_biquad, SR=24000, cutoff=8000, Q=0.707)
over wav [64, 480000], data-parallel across 8 TRN2 NeuronCores.

The biquad's poles sit at |z| = sqrt(a2) ~= 0.49, so the IIR is numerically an
8-tap causal FIR at the 2e-2 tolerance (tail energy beyond tap 8 is 5.8e-3).
The kernel boundary is bf16: the host rounds wav to bf16 before upload and
upcasts the bf16 result, halving HBM traffic (15.4 MB/core, ~44 us DMA floor).

Layout per core: 8 rows x 480000 = 128 chunks of 30000 samples, one per SBUF
partition. Time is cut into 120-sample slices; each slice's 128-sample window
(8-sample halo + 120 new) is PE-transposed so window-time sits on partitions,
then one independent start=stop=True matmul against the banded H' [128, 120]
(H'[w, n] = h[n+8-w]) produces the slice's output. No carry matmuls, no PSUM
accumulation overlap: chunk continuity is the DMA halo (8 samples re-read
before each group; zeros at row starts). PSUM->SBUF slab copies and output
drains alternate between the Act and DVE engines; input DMAs ride the sync
(SP) ring, output DMAs the gpsimd SWDGE path. Emission is software-pipelined
one sub-iteration ahead so the PE never waits on a slab copy.
"""

import sys

sys.path.insert(0, "/opt/trn_rl_repo")

import numpy as np
import ml_dtypes

import concourse.mybir as mybir
import concourse.tile as tile
from concourse import bacc
from concourse.bass_utils import run_bass_kernel_spmd

f32 = mybir.dt.float32
bf16 = mybir.dt.bfloat16

# ---- problem constants ----------------------------------------------------
SR = 24000
CUTOFF = 8000.0
Q = 0.707

B_FULL, T = 64, 480000
N_CORES = 8
R = B_FULL // N_CORES          # rows per core
NCH = 16                       # chunks per row
P = R * NCH                    # 128 partitions (one chunk each)
L = T // NCH                   # 30000 samples per chunk
LS = 120                       # slice length
D = 8                          # FIR taps (window = LS + D = 128 partitions)
W = LS + D                     # 128: transpose window rows
NSL = L // LS                  # 250 slices per chunk
GS = 16                        # slices per DMA group
GW = GS * LS                   # 1920 samples per group
SUB = 4                        # slices per PSUM sub-iteration


def _fir_taps():
    w0 = 2.0 * np.pi * CUTOFF / SR
    alpha = np.sin(w0) / (2.0 * Q)
    cos_w0 = np.cos(w0)
    b0 = (1.0 - cos_w0) / 2.0
    b1 = 1.0 - cos_w0
    b2 = b0
    a0 = 1.0 + alpha
    a1 = -2.0 * cos_w0
    a2 = 1.0 - alpha
    b0, b1, b2, a1, a2 = (np.float32(b0 / a0), np.float32(b1 / a0),
                          np.float32(b2 / a0), np.float32(a1 / a0),
                          np.float32(a2 / a0))
    h = np.zeros(D, dtype=np.float64)
    x1 = x2 = y1 = y2 = 0.0
    for t in range(D):
        x = 1.0 if t == 0 else 0.0
        y = (float(b0) * x + float(b1) * x1 + float(b2) * x2
             - float(a1) * y1 - float(a2) * y2)
        h[t] = y
        x2, x1 = x1, x
        y2, y1 = y1, y
    return h


def _const_blk():
    h = _fir_taps()
    # H'[w, n] = h[n + D - w]: window row w contributes tap d = n + D - w to
    # output sample n of the slice.
    Hp = np.zeros((W, LS), dtype=np.float32)
    for w in range(W):
        for n in range(LS):
            d = n + D - w
            if 0 <= d < D:
                Hp[w, n] = h[d]
    blk = np.concatenate([Hp, np.eye(W, dtype=np.float32)], axis=1)
    return blk.astype(ml_dtypes.bfloat16)   # [128, 120 + 128]


def _build():
    CONST_np = _const_blk()
    nc = bacc.Bacc("TRN2", target_bir_lowering=False)

    wav = nc.dram_tensor("wav", [R, T], bf16, kind="ExternalInput")
    out = nc.dram_tensor("out", [R, T], bf16, kind="ExternalOutput")
    const_d = nc.inline_tensor(CONST_np, name="hconst")
    zeros_d = nc.inline_tensor(
        np.zeros((1, D), dtype=ml_dtypes.bfloat16), name="zeros8")

    wav_ch = wav[:, :].rearrange("r (c l) -> (r c) l", c=NCH)   # [128, 30000]
    out_ch = out[:, :].rearrange("r (c l) -> (r c) l", c=NCH)

    # groups of GS slices; last group has the remainder
    groups = []
    done = 0
    while done < NSL:
        n = min(GS, NSL - done)
        groups.append((done, n))
        done += n

    with tile.TileContext(nc) as tc:
        with (
            tc.tile_pool(name="const", bufs=1) as cpool,
            tc.tile_pool(name="io", bufs=4) as iopool,
            tc.tile_pool(name="work", bufs=4) as wpool,
            tc.tile_pool(name="psum", bufs=4, space="PSUM") as ppool,
        ):
            cblk = cpool.tile([W, LS + W], bf16)
            nc.sync.dma_start(cblk[:], const_d[:, :])
            Hp = cblk[:, 0:LS]
            ident = cblk[:, LS:]

            engs = (nc.scalar, nc.vector)
            copies = (lambda o, i: nc.scalar.copy(o, i),
                      lambda o, i: nc.vector.tensor_copy(o, i))

            # software pipeline state: the previous sub-iteration's slab,
            # whose matmuls + drain are emitted after the next sub-iter's
            # transposes.
            pend = None          # (slab, s, si, yout, flush)
            sub_idx = 0

            def emit_tail(pend):
                slab, s, si, yout, flush, ci = pend
                py = ppool.tile([P, SUB * LS], f32, tag="py")
                for j in range(s):
                    nc.tensor.matmul(
                        py[:, j * LS: (j + 1) * LS],
                        slab[:, j * P: (j + 1) * P],
                        Hp,
                        start=True, stop=True, skip_group_check=True,
                    )
                copies[1 - ci](yout[:, si * LS: (si + s) * LS], py[:, : s * LS])
                if flush is not None:
                    gbase, gw, yt = flush
                    nc.gpsimd.dma_start(out_ch[:, gbase: gbase + gw], yt[:, :gw])

            for gi, (sl0, nsl) in enumerate(groups):
                gbase = sl0 * LS
                gw = nsl * LS

                xin = iopool.tile([P, D + GW], bf16, tag="xin")
                if gi == 0:
                    # first group: halo = previous chunk's last D samples via a
                    # partition-shifted DMA. Row-start partitions get the
                    # previous row's tail — finite but wrong; their first D
                    # output samples are recomputed exactly on the host.
                    # Partition 0's halo is zeroed from DRAM (uninitialized
                    # SBUF could hold NaN, and NaN*0 poisons the matmul).
                    nc.sync.dma_start(xin[:, D: D + gw], wav_ch[:, 0:gw])
                    nc.scalar.dma_start(xin[1:P, 0:D], wav_ch[0: P - 1, L - D: L])
                    nc.scalar.dma_start(xin[0:1, 0:D], zeros_d[:, :])
                else:
                    nc.sync.dma_start(
                        xin[:, 0: D + gw],
                        wav_ch[:, gbase - D: gbase + gw],
                    )
                yout = iopool.tile([P, GW], bf16, tag="yout")

                si = 0
                while si < nsl:
                    s = min(SUB, nsl - si)
                    pt = ppool.tile([W, SUB * P], bf16, tag="pt")
                    for j in range(s):
                        k = si + j
                        nc.tensor.transpose(
                            pt[:, j * P: (j + 1) * P],
                            xin[:, k * LS: k * LS + W],
                            ident,
                        )
                    slab = wpool.tile([W, SUB * P], bf16, tag="slab")
                    ci = sub_idx % 2
                    copies[ci](slab[:, : s * P], pt[:, : s * P])

                    if pend is not None:
                        emit_tail(pend)
                    flush = (gbase, gw, yout) if si + s >= nsl else None
                    pend = (slab, s, si, yout, flush, ci)
                    sub_idx += 1
                    si += s

            emit_tail(pend)

    nc.finalize()
    return nc


def _patch_warmup(out: np.ndarray, wav: np.ndarray):
    """Each waveform's first D samples start from zero filter state; the
    device computed them with a bogus halo. Run the exact IIR recurrence for
    those D samples on the host."""
    w0 = 2.0 * np.pi * CUTOFF / SR
    alpha = np.sin(w0) / (2.0 * Q)
    cos_w0 = np.cos(w0)
    a0 = 1.0 + alpha
    b0 = np.float32((1.0 - cos_w0) / 2.0 / a0)
    b1 = np.float32((1.0 - cos_w0) / a0)
    b2 = np.float32((1.0 - cos_w0) / 2.0 / a0)
    a1 = np.float32(-2.0 * cos_w0 / a0)
    a2 = np.float32((1.0 - alpha) / a0)
    x = wav[:, :D].astype(np.float64)
    B = x.shape[0]
    x1 = np.zeros(B); x2 = np.zeros(B)
    y1 = np.zeros(B); y2 = np.zeros(B)
    for t in range(D):
        xt = x[:, t]
        yt = b0 * xt + b1 * x1 + b2 * x2 - a1 * y1 - a2 * y2
        out[:, t] = yt.astype(np.float32)
        x2, x1 = x1, xt
        y2, y1 = y1, yt


_NC_CACHE = None


def _get_nc():
    global _NC_CACHE
    if _NC_CACHE is None:
        _NC_CACHE = _build()
    return _NC_CACHE


def _run(wav_full: np.ndarray, trace: bool = False):
    global _NC_CACHE
    wav_full = np.ascontiguousarray(wav_full, dtype=np.float32)
    wav_bf = wav_full.astype(ml_dtypes.bfloat16)
    in_maps = [
        {"wav": wav_bf[i * R: (i + 1) * R]} for i in range(N_CORES)
    ]
    last_err = None
    for attempt in range(3):
        try:
            res = run_bass_kernel_spmd(
                _get_nc(), in_maps, core_ids=list(range(N_CORES)), trace=trace
            )
            out = np.concatenate(
                [np.asarray(res.results[i]["out"]) for i in range(N_CORES)],
                axis=0).astype(np.float32)
            _patch_warmup(out, wav_full)
            return out, res
        except Exception as e:          # transient device errors recover on retry
            last_err = e
            _NC_CACHE = None
            try:
                import jax
                jax.clear_caches()
            except Exception:
                pass
            import time
            time.sleep(5 * (attempt + 1))
    raise last_err


def kernel(wav: np.ndarray) -> np.ndarray:
    out, _ = _run(np.asarray(wav))
    return out
